# revision 29
# baseline (speedup 1.0000x reference)
"""MoE (group-limited top-k routing) Trainium2 kernel, expert-parallel on 8 cores.

Strategy:
  - Host (numpy): gate softmax + group-limited top-4 routing (control plane,
    ~0.06% of FLOPs), token dispatch (gather per expert) and final combine.
  - Device (8 NeuronCores, SPMD): every core owns two routed experts — slot 0
    gets one of the 8 most-loaded experts (capacity cap0), slot 1 one of the 8
    least-loaded (cap1 <= cap0) — so padding tracks the real count
    distribution instead of the global max.
  - Routed experts run in fp8 e4m3 with DoubleRow perf mode (2 contraction
    k-tiles per matmul at 0.5 cycles/row). Weights are pre-scaled on host
    (W1*64, W3*8, W2*64) to dodge e4m3's tiny subnormal range; the 1/64 comes
    out inside the silu (activation scale) and the rest (1/512) is folded
    into the gate-weight epilogue. h is stored fp8 (= 8*h_true, max ~104,
    well inside e4m3's +-448). Routed error is diluted ~5x because the
    shared expert carries ~98% of the output norm; measured total rel err
    ~1.4e-2 against the 2e-2 gate.
  - The shared expert (carrying the norm) runs in fp16 (same PE rate as
    bf16, 8x finer mantissa), inter-dim sharded (2816/8=352, padded to 384);
    each core computes a partial z for all 2048 tokens; host sums partials.
  - Weight/token loads stream on the SP HWDGE queue set; output stores and
    shared-token loads go on the Activation HWDGE set so stores can never
    head-block the weight stream (the 16 queues per set are FIFO).
  - Phase order: the two routed experts (weights streamed, read once), then
    the shared expert (weights resident in SBUF, read once).
"""

import numpy as np
import ml_dtypes

F8 = ml_dtypes.float8_e4m3   # matches mybir.dt.np(float8e4); NO saturation --
F16 = np.float16             # host must clip before casting to F8

# Model dims (hardcoded per problem spec nn_MoE_51616916963811)
D = 2048
INTER = 1408
E = 16
TOPK = 4
G = 4
TOPK_G = 2
T = 2048
SI = 2816           # shared inter dim
SI_SHARD = SI // 8  # 352
SI_PAD = 384        # padded to 3x128
ROUTE_SCALE = 1.0

NCORES = 8
ELOC = 2            # experts per core
TCHUNK = 512        # shared-expert token chunk
KD = D // 128       # 16 contraction chunks over D
KI = INTER // 128   # 11 tiles over INTER
KS = SI_PAD // 128  # 3 tiles over padded shared inter

A1 = 64.0           # host pre-scale on W1 (silu divides it back out)
A3 = 8.0            # host pre-scale on W3 (folded into gw epilogue)
A2 = 64.0           # host pre-scale on W2 (folded into gw epilogue)
F8CLIP = 440.0      # e4m3 overflows to inf past 448 -- clip before cast

CAP_MIN = 512       # capacity floor (expected count is exactly 512)
MAX_CAP = 704       # capacity ceiling (SBUF budget)

_CACHE = {}


def _roundup32(n):
    return ((int(n) + 31) // 32) * 32


def _chunks_of(cap):
    """Moving-dim chunks (PSUM banks are 512 fp32 wide)."""
    if cap <= 512:
        return (cap,)
    half = _roundup32((cap + 1) // 2)
    assert half <= 512
    return (half, cap - half)


def _pick_caps(counts):
    """Per-slot capacities from the sorted per-expert counts.

    Slot 0 holds the 8 largest experts (cap0 = max count), slot 1 the 8
    smallest (cap1 = 9th largest). Rounded to 32; clamped to
    [CAP_MIN, MAX_CAP] — kernel() truncates the (never expected) overflow
    tokens by lowest gate weight."""
    srt = np.sort(counts)[::-1]
    cap0 = max(CAP_MIN, min(MAX_CAP, _roundup32(srt[0])))
    cap1 = max(CAP_MIN, min(cap0, _roundup32(srt[NCORES])))
    return cap0, cap1


# ---------------------------------------------------------------- host gate --
def _route(x2d, Wg):
    """Replicates the reference gate in numpy float32.

    Returns topi [T, TOPK] int64 and weights [T, TOPK] float32."""
    logits = x2d.astype(np.float32) @ Wg.T.astype(np.float32)      # [T, E]
    m = logits.max(axis=-1, keepdims=True)
    ex = np.exp(logits - m)
    scores = ex / ex.sum(axis=-1, keepdims=True)                   # [T, E]
    sg = scores.reshape(T, G, E // G)
    gs = sg.max(axis=-1)                                           # [T, G]
    gidx = np.argsort(-gs, axis=1, kind="stable")[:, :TOPK_G]
    gmask = np.zeros((T, G), dtype=bool)
    np.put_along_axis(gmask, gidx, True, axis=1)
    masked = np.where(gmask[:, :, None], sg, -np.inf).reshape(T, E)
    topi = np.argsort(-masked, axis=1, kind="stable")[:, :TOPK]
    weights = np.take_along_axis(scores, topi, axis=1) * ROUTE_SCALE
    return topi, weights.astype(np.float32)


# ------------------------------------------------------------ host packing --
def _q8(a):
    return np.clip(a, -F8CLIP, F8CLIP).astype(F8)


def _tile_kxm4(w, scale):
    """[R, C] weight -> lhsT tiles [R/128, 128(p), C/128, 128] fp8 where
    tile[i, p, ko, m] = scale * w[i*128+m, ko*128+p]."""
    R, C = w.shape
    ri, ci = R // 128, C // 128
    return _q8(np.ascontiguousarray(
        (w * scale).reshape(ri, 128, ci, 128).transpose(0, 3, 2, 1)))


def _tile_kxm(w):
    """[R, C] weight -> lhsT tiles [R/128, 128(p), C/128 * 128] fp16."""
    R, C = w.shape
    ri, ci = R // 128, C // 128
    return np.ascontiguousarray(
        w.reshape(ri, 128, ci, 128).transpose(0, 3, 2, 1)
    ).reshape(ri, 128, ci * 128).astype(F16)


def _tile_xT(xrows, cap):
    """[n, D] activations -> [128(p), KD, cap] fp8, zero-padded to cap."""
    n = xrows.shape[0]
    out = np.zeros((128, KD, cap), dtype=np.float32)
    xt = xrows.T.reshape(KD, 128, n).transpose(1, 0, 2)  # [128, KD, n]
    out[:, :, :n] = xt
    return _q8(out)


# ------------------------------------------------------------- bass kernel --
def _build_nc(caps):
    import concourse.bass as bass
    import concourse.tile as tile
    from concourse import bacc, mybir

    f32 = mybir.dt.float32
    f16 = mybir.dt.float16
    f8 = mybir.dt.float8e4
    DR = mybir.MatmulPerfMode.DoubleRow
    AF = mybir.ActivationFunctionType

    cap0, cap1 = caps
    CHK = [_chunks_of(cap0), _chunks_of(cap1)]
    OFF = []
    for chunks in CHK:
        off, lst = 0, []
        for w in chunks:
            lst.append((off, w))
            off += w
        OFF.append(lst)
    MAXW = max(max(c) for c in CHK)
    NCT = T // TCHUNK
    KP = KD // 2        # 8 contraction k-tile pairs over D
    IP = KI // 2        # 5 full pairs over INTER (plus one odd tile)

    nc = bacc.Bacc("TRN2", target_bir_lowering=False, debug=False,
                   enable_asserts=False)

    # Inputs (per core). Routed operands fp8 (DoubleRow), shared fp16.
    xg = nc.dram_tensor("xg", [ELOC, 128, KD, cap0], f8, kind="ExternalInput").ap()
    gw = nc.dram_tensor("gw", [ELOC, 128, cap0], f32, kind="ExternalInput").ap()
    w1 = nc.dram_tensor("w1", [ELOC, KI, 128, KD, 128], f8, kind="ExternalInput").ap()
    w3 = nc.dram_tensor("w3", [ELOC, KI, 128, KD, 128], f8, kind="ExternalInput").ap()
    w2 = nc.dram_tensor("w2", [ELOC, KD, 128, KI, 128], f8, kind="ExternalInput").ap()
    xt = nc.dram_tensor("xt", [T // TCHUNK, 128, KD, TCHUNK], f16, kind="ExternalInput").ap()
    ws1 = nc.dram_tensor("ws1", [KS, 128, KD * 128], f16, kind="ExternalInput").ap()
    ws3 = nc.dram_tensor("ws3", [KS, 128, KD * 128], f16, kind="ExternalInput").ap()
    ws2 = nc.dram_tensor("ws2", [KD, 128, KS * 128], f16, kind="ExternalInput").ap()
    # Outputs (layouts chosen so stores batch into few huge DMAs: the
    # in-order issuing engines pay ~600ns per DMA instruction, so frequent
    # small stores would pace the PSUM-drain path)
    yt = nc.dram_tensor("yt", [ELOC, 128, KD, cap0], f16, kind="ExternalOutput").ap()
    zt = nc.dram_tensor("zt", [T // TCHUNK, 2, 128, KD // 2, TCHUNK], f16,
                        kind="ExternalOutput").ap()

    with tile.TileContext(nc) as tc:
        # All pools coexist (~140KB of 208KB SBUF) so resident tiles never
        # WAR-wait on streamed ones: a dma_start with an unmet wait stalls
        # the whole in-order SP issue stream behind it.
        wg12 = tc.alloc_tile_pool(name="wg12", bufs=3)
        xs = tc.alloc_tile_pool(name="xs", bufs=3)
        pg12 = tc.alloc_tile_pool(name="pg12", bufs=2, space="PSUM")
        pg3 = tc.alloc_tile_pool(name="pg3", bufs=4, space="PSUM")
        shres = tc.alloc_tile_pool(name="shres", bufs=1)   # shared residents
        wg3 = tc.alloc_tile_pool(name="wg3", bufs=8)
        htp = tc.alloc_tile_pool(name="htp", bufs=1)
        gwp = tc.alloc_tile_pool(name="gwp", bufs=2)
        actp = tc.alloc_tile_pool(name="actp", bufs=3)
        stg = tc.alloc_tile_pool(name="stg", bufs=2)
        hstp = tc.alloc_tile_pool(name="hstp", bufs=2)
        acts = tc.alloc_tile_pool(name="acts", bufs=3)
        stgs = tc.alloc_tile_pool(name="stgs", bufs=2)

        ws1r = [None] * KS
        ws3r = [None] * KS
        ws2r = [None, None]
        xt_tiles = {}

        # ---------------- routed experts, weights streamed ----------------
        for s in range(ELOC):
            cap = caps[s]
            coff = OFF[s]
            if s == 0:
                # startup order: first i-tile weights, then tokens in
                # ko-quarters (contiguous lines; matmul 0 starts after the
                # first quarter)
                w1t0 = wg12.tile([128, KD, 128], f8, tag="w1t", name="w1t0_0")
                nc.sync.dma_start(w1t0[:], w1[0, 0])
                xg_s = xs.tile([128, KD, cap0], f8, tag="x", name="xg0")
                for kq in range(0, KD, 4):
                    nc.sync.dma_start(xg_s[:, kq:kq + 4, :cap],
                                      xg[0, :, kq:kq + 4, :cap])
                w3t0 = wg12.tile([128, KD, 128], f8, tag="w3t", name="w3t0_0")
                nc.sync.dma_start(w3t0[:], w3[0, 0])
                gw_s = gwp.tile([128, cap0], f32, tag="gw", name="gw0")
                nc.sync.dma_start(gw_s[:, :cap], gw[0, :, :cap])
            else:
                xg_s, gw_s = xg_next, gw_next

            ht = htp.tile([128, KI, cap0], f8, tag="ht", name=f"ht{s}")

            # GEMM1/2 (DoubleRow): hT[i, c] = silu(x @ W1^T) * (x @ W3^T)
            for i in range(KI):
                if s == 0 and i == 0:
                    w1t, w3t = w1t0, w3t0
                else:
                    w1t = wg12.tile([128, KD, 128], f8, tag="w1t", name=f"w1t{s}_{i}")
                    nc.sync.dma_start(w1t[:], w1[s, i])
                    w3t = wg12.tile([128, KD, 128], f8, tag="w3t", name=f"w3t{s}_{i}")
                    nc.sync.dma_start(w3t[:], w3[s, i])
                # chunk-inner loops: consecutive matmuls reuse the same
                # stationary k-pair, so narrow-chunk DR runs matmul-limited
                # (122ns) instead of LDWEIGHTS-limited (135ns)
                p1s = [pg12.tile([128, MAXW], f32, tag="p1", name="p1")
                       for _ in coff]
                p3s = [pg12.tile([128, MAXW], f32, tag="p3", name="p3")
                       for _ in coff]
                for kp in range(KP):
                    for ci, (c0, cw) in enumerate(coff):
                        nc.tensor.matmul(
                            p1s[ci][:, :cw], w1t[:, 2 * kp:2 * kp + 2, :],
                            xg_s[:, 2 * kp:2 * kp + 2, c0:c0 + cw],
                            start=(kp == 0), stop=(kp == KP - 1), perf_mode=DR)
                for kp in range(KP):
                    for ci, (c0, cw) in enumerate(coff):
                        nc.tensor.matmul(
                            p3s[ci][:, :cw], w3t[:, 2 * kp:2 * kp + 2, :],
                            xg_s[:, 2 * kp:2 * kp + 2, c0:c0 + cw],
                            start=(kp == 0), stop=(kp == KP - 1), perf_mode=DR)
                for ci, (c0, cw) in enumerate(coff):
                    a1 = actp.tile([128, MAXW], f16, tag="act", name="a1")
                    nc.scalar.activation(a1[:, :cw], p1s[ci][:, :cw], AF.Silu,
                                         scale=1.0 / A1)
                    nc.vector.tensor_mul(ht[:, i, c0:c0 + cw], a1[:, :cw],
                                         p3s[ci][:, :cw])
                # shared-expert residents + first token chunks stream on the
                # Act queue during s1-G12 (it is empty there), touching
                # neither the SP weight stream nor the G3 store path
                if s == 1 and i < 6:
                    j, which = divmod(i, 2)
                    if which == 0:
                        ws1r[j] = shres.tile([128, KD * 128], f16, tag=f"ws1r{j}",
                                             name=f"ws1r{j}")
                        nc.scalar.dma_start(ws1r[j][:], ws1[j])
                    else:
                        ws3r[j] = shres.tile([128, KD * 128], f16, tag=f"ws3r{j}",
                                             name=f"ws3r{j}")
                        nc.scalar.dma_start(ws3r[j][:], ws3[j])
                if s == 1 and i in (6, 7):
                    j = i - 6
                    ws2r[j] = shres.tile([128, KD // 2, KS * 128], f16,
                                         tag=f"ws2r{j}", name=f"ws2r{j}")
                    for dd in range(KD // 2):
                        nc.scalar.dma_start(ws2r[j][:, dd], ws2[j * (KD // 2) + dd])
                if s == 1 and i in (8, 9):
                    ct0 = i - 8
                    xt_tiles[ct0] = xs.tile([128, KD, TCHUNK], f16, tag="x",
                                            name=f"xt{ct0}")
                    nc.scalar.dma_start(xt_tiles[ct0][:], xt[ct0])

            # GEMM3 (DoubleRow + one odd tile): yT = (hT^T @ W2^T)^T * gate.
            # Epilogue products collect in one staging tile; two big DMAs
            # per expert instead of 2 per d keep the issue engines off the
            # critical path.
            ytst = stg.tile([128, KD, cap0], f16, tag="st", name=f"ytst{s}")
            for d in range(KD):
                w2t = wg3.tile([128, KI, 128], f8, tag="w2t", name=f"w2t{s}_{d}")
                nc.sync.dma_start(w2t[:], w2[s, d])
                # next expert's tokens prefetch rides s0-G3's SP slack (the
                # w2 stream only needs ~100GB/s); both destination slots are
                # fresh, so these never stall the in-order SP issue stream
                if s == 0 and d in (2, 4, 6, 8):
                    kq = 4 * (d // 2 - 1)
                    if d == 2:
                        xg_next = xs.tile([128, KD, cap0], f8, tag="x", name="xg1")
                    nc.sync.dma_start(xg_next[:, kq:kq + 4, :caps[1]],
                                      xg[1, :, kq:kq + 4, :caps[1]])
                    if d == 8:
                        gw_next = gwp.tile([128, cap0], f32, tag="gw", name="gw1")
                        nc.sync.dma_start(gw_next[:, :caps[1]], gw[1, :, :caps[1]])
                for (c0, cw) in coff:
                    py = pg3.tile([128, MAXW], f32, tag="py", name="py")
                    for ip in range(IP):
                        nc.tensor.matmul(
                            py[:, :cw], w2t[:, 2 * ip:2 * ip + 2, :],
                            ht[:, 2 * ip:2 * ip + 2, c0:c0 + cw],
                            start=(ip == 0), stop=False, perf_mode=DR)
                    nc.tensor.matmul(
                        py[:, :cw], w2t[:, KI - 1, :],
                        ht[:, KI - 1, c0:c0 + cw],
                        start=False, stop=True)
                    nc.vector.tensor_mul(ytst[:, d, c0:c0 + cw], py[:, :cw],
                                         gw_s[:, c0:c0 + cw])
                if d == KD // 2 - 1:
                    nc.scalar.dma_start(yt[s, :, :KD // 2, :cap],
                                        ytst[:, :KD // 2, :cap])
                if d == KD - 1:
                    nc.scalar.dma_start(yt[s, :, KD // 2:, :cap],
                                        ytst[:, KD // 2:, :cap])

        # -------- shared expert (inter-sharded, fp16), weights resident ----
        # software pipeline: G1(ct) then G2(ct-1), so the PSUM drain /
        # vector-mul tail of hst(ct) hides behind the next chunk's matmuls
        def g1(ct):
            # SP is idle in the shared phase: xt loads and zt stores go
            # there, leaving the scalar engine free for silu + PSUM drains
            if ct + 2 < NCT:
                xt_tiles[ct + 2] = xs.tile([128, KD, TCHUNK], f16, tag="x",
                                           name=f"xt{ct + 2}")
                nc.sync.dma_start(xt_tiles[ct + 2][:], xt[ct + 2])
            xt_c = xt_tiles.pop(ct)
            hst = hstp.tile([128, KS, TCHUNK], f16, tag="hst", name=f"hst{ct}")
            for i in range(KS):
                p1 = pg12.tile([128, TCHUNK], f32, tag="p1", name="p1")
                p3 = pg12.tile([128, TCHUNK], f32, tag="p3", name="p3")
                for ko in range(KD):
                    nc.tensor.matmul(
                        p1[:], ws1r[i][:, ko * 128:(ko + 1) * 128],
                        xt_c[:, ko, :],
                        start=(ko == 0), stop=(ko == KD - 1))
                for ko in range(KD):
                    nc.tensor.matmul(
                        p3[:], ws3r[i][:, ko * 128:(ko + 1) * 128],
                        xt_c[:, ko, :],
                        start=(ko == 0), stop=(ko == KD - 1))
                a1 = acts.tile([128, TCHUNK], f16, tag="acts", name="a1")
                nc.scalar.activation(a1[:], p1[:], AF.Silu)
                nc.vector.tensor_mul(hst[:, i], a1[:], p3[:])
            return hst

        def g2(ct, hst):
            for half in range(2):
                ztst = stgs.tile([128, KD // 2, TCHUNK], f16, tag="stgs",
                                 name=f"ztst{ct}_{half}")
                for dd in range(KD // 2):
                    d = half * (KD // 2) + dd
                    py = pg3.tile([128, TCHUNK], f32, tag="py", name="py")
                    w2half = ws2r[half]
                    for io in range(KS):
                        nc.tensor.matmul(
                            py[:], w2half[:, dd, io * 128:(io + 1) * 128],
                            hst[:, io],
                            start=(io == 0), stop=(io == KS - 1))
                    # alternate the PSUM->SBUF drain across two engines so
                    # it keeps up with the 3-matmul fill
                    if d % 2 == 0:
                        nc.scalar.activation(ztst[:, dd], py[:], AF.Copy)
                    else:
                        nc.vector.tensor_copy(ztst[:, dd], py[:])
                nc.sync.dma_start(zt[ct, half], ztst[:])

        hst_prev = None
        for ct in range(NCT):
            hst = g1(ct)
            if hst_prev is not None:
                g2(ct - 1, hst_prev)
            hst_prev = hst
        g2(NCT - 1, hst_prev)

        stgs.release()
        acts.release()
        hstp.release()
        stg.release()
        actp.release()
        gwp.release()
        htp.release()
        wg3.release()
        shres.release()
        pg3.release()
        pg12.release()
        xs.release()
        wg12.release()

    nc.compile()
    return nc


def _get_nc(caps):
    key = ("nc", caps)
    if key not in _CACHE:
        _CACHE[key] = _build_nc(caps)
    return _CACHE[key]


# ------------------------------------------------------------------ kernel --
def kernel(x, Wg, W1, W3, W2, Ws1, Ws3, Ws2):
    from concourse.bass_utils import run_bass_kernel_spmd

    x = np.asarray(x, dtype=np.float32)
    x2d = np.ascontiguousarray(x.reshape(T, D))
    Wg = np.asarray(Wg, dtype=np.float32)
    W1 = np.asarray(W1, dtype=np.float32)
    W3 = np.asarray(W3, dtype=np.float32)
    W2 = np.asarray(W2, dtype=np.float32)
    Ws1 = np.asarray(Ws1, dtype=np.float32)
    Ws3 = np.asarray(Ws3, dtype=np.float32)
    Ws2 = np.asarray(Ws2, dtype=np.float32)

    # ---- host routing + dispatch ----
    topi, weights = _route(x2d, Wg)
    flat_e = topi.ravel()
    flat_t = np.repeat(np.arange(T), TOPK)
    flat_w = weights.ravel()
    order = np.argsort(flat_e, kind="stable")
    se, st_, sw = flat_e[order], flat_t[order], flat_w[order]
    bounds = np.searchsorted(se, np.arange(E + 1))
    tok_of = [st_[bounds[e]:bounds[e + 1]] for e in range(E)]
    wt_of = [sw[bounds[e]:bounds[e + 1]] for e in range(E)]
    counts = np.array([len(t) for t in tok_of])

    caps = _pick_caps(counts)
    cap0, cap1 = caps
    # expert -> (core, slot): slot 0 gets the 8 largest, slot 1 the 8
    # smallest (largest paired with smallest so per-core work is even)
    ranks = np.argsort(-counts, kind="stable")
    assign = [[int(ranks[c]), int(ranks[2 * NCORES - 1 - c])]
              for c in range(NCORES)]

    truncated = False
    for e in range(E):
        cap_e = cap0 if any(a[0] == e for a in assign) else cap1
        if len(tok_of[e]) > cap_e:  # only possible at the MAX_CAP clamp
            keep = np.sort(np.argsort(-wt_of[e], kind="stable")[:cap_e])
            tok_of[e] = tok_of[e][keep]
            wt_of[e] = wt_of[e][keep]
            truncated = True

    # ---- build per-core input maps ----
    xt_full = x2d.T.reshape(KD, 128, T).transpose(1, 0, 2)  # [128, KD, T]
    xt_tiles = np.ascontiguousarray(
        np.stack([xt_full[:, :, i * TCHUNK:(i + 1) * TCHUNK]
                  for i in range(T // TCHUNK)])).astype(F16)

    in_maps = []
    for c in range(NCORES):
        exps = assign[c]
        xg_c = np.stack([_tile_xT(x2d[tok_of[e]], cap0) for e in exps])
        # epilogue scale: py = (A2*W2)@(A3*h) => multiply by gw/(A2*A3)
        gw_c = np.zeros((ELOC, 128, cap0), dtype=np.float32)
        for s, e in enumerate(exps):
            gw_c[s, :, :len(wt_of[e])] = wt_of[e][None, :] / (A2 * A3)
        w1_c = np.stack([_tile_kxm4(W1[e], A1) for e in exps])
        w3_c = np.stack([_tile_kxm4(W3[e], A3) for e in exps])
        w2_c = np.stack([_tile_kxm4(W2[e], A2) for e in exps])

        lo = c * SI_SHARD
        ws1_s = np.zeros((SI_PAD, D), dtype=np.float32)
        ws1_s[:SI_SHARD] = Ws1[lo:lo + SI_SHARD]
        ws3_s = np.zeros((SI_PAD, D), dtype=np.float32)
        ws3_s[:SI_SHARD] = Ws3[lo:lo + SI_SHARD]
        ws2_s = np.zeros((D, SI_PAD), dtype=np.float32)
        ws2_s[:, :SI_SHARD] = Ws2[:, lo:lo + SI_SHARD]

        in_maps.append({
            "xg": xg_c, "gw": gw_c, "w1": w1_c, "w3": w3_c, "w2": w2_c,
            "xt": xt_tiles,
            "ws1": _tile_kxm(ws1_s), "ws3": _tile_kxm(ws3_s),
            "ws2": _tile_kxm(ws2_s),
        })

    # ---- run on 8 cores ----
    shapes = {
        "xg": ((ELOC, 128, KD, cap0), F8), "gw": ((ELOC, 128, cap0), np.float32),
        "w1": ((ELOC, KI, 128, KD, 128), F8), "w3": ((ELOC, KI, 128, KD, 128), F8),
        "w2": ((ELOC, KD, 128, KI, 128), F8),
        "xt": ((T // TCHUNK, 128, KD, TCHUNK), F16),
        "ws1": ((KS, 128, KD * 128), F16), "ws3": ((KS, 128, KD * 128), F16),
        "ws2": ((KD, 128, KS * 128), F16),
    }
    for m in in_maps:
        for k, v in m.items():
            assert v.shape == shapes[k][0], (k, v.shape, shapes[k][0])
            assert v.dtype == shapes[k][1], (k, v.dtype)

    nc = _get_nc(caps)
    res = run_bass_kernel_spmd(nc, in_maps, core_ids=list(range(NCORES)))
    _CACHE["last_results"] = res

    # ---- combine on host ----
    # routed: yt[s, d, p, c] = w * Y[c, d*128+p]
    cat_tok = []
    cat_rows = []
    for c in range(NCORES):
        ytc = res.results[c]["yt"]  # [ELOC, 128, KD, cap0] fp16
        for s in range(ELOC):
            e = assign[c][s]
            n = len(tok_of[e])
            rows = ytc[s].transpose(2, 1, 0).reshape(cap0, D)[:n].astype(
                np.float32)  # [n, D]
            cat_tok.append(tok_of[e])
            cat_rows.append(rows)
    cat_tok = np.concatenate(cat_tok)
    cat_rows = np.concatenate(cat_rows, axis=0)
    if truncated:
        y = np.zeros((T, D), dtype=np.float32)
        np.add.at(y, cat_tok, cat_rows)
    else:
        # every token has exactly TOPK contributions
        order = np.argsort(cat_tok, kind="stable")
        y = cat_rows[order].reshape(T, TOPK, D).sum(axis=1)

    # shared: sum partials; zt[ct, half, p, dd, t'] = Z[t, d*128+p] with
    # d = half*8+dd, t = ct*TCHUNK+t'
    z_acc = res.results[0]["zt"].astype(np.float32)
    for c in range(1, NCORES):
        z_acc += res.results[c]["zt"].astype(np.float32)
    z = z_acc.transpose(1, 3, 2, 0, 4).reshape(D, T).T  # [T, D]

    return (y + z).reshape(1, T, D).astype(np.float32)


# revision 31
# speedup vs baseline: 1.0028x; 1.0028x over previous
"""MoE (group-limited top-k routing) Trainium2 kernel, expert-parallel on 8 cores.

Strategy:
  - Host (numpy): gate softmax + group-limited top-4 routing (control plane,
    ~0.06% of FLOPs), token dispatch (gather per expert) and final combine.
  - Device (8 NeuronCores, SPMD): every core owns two routed experts — slot 0
    gets one of the 8 most-loaded experts (capacity cap0), slot 1 one of the 8
    least-loaded (cap1 <= cap0) — so padding tracks the real count
    distribution instead of the global max.
  - Routed experts run in fp8 e4m3 with DoubleRow perf mode (2 contraction
    k-tiles per matmul at 0.5 cycles/row). Weights are pre-scaled on host
    (W1*64, W3*8, W2*64) to dodge e4m3's tiny subnormal range; the 1/64 comes
    out inside the silu (activation scale) and the rest (1/512) is folded
    into the gate-weight epilogue. h is stored fp8 (= 8*h_true, max ~104,
    well inside e4m3's +-448). Routed error is diluted ~5x because the
    shared expert carries ~98% of the output norm; measured total rel err
    ~1.4e-2 against the 2e-2 gate.
  - The shared expert (carrying the norm) runs in fp16 (same PE rate as
    bf16, 8x finer mantissa), inter-dim sharded (2816/8=352, padded to 384);
    each core computes a partial z for all 2048 tokens; host sums partials.
  - Weight/token loads stream on the SP HWDGE queue set; output stores and
    shared-token loads go on the Activation HWDGE set so stores can never
    head-block the weight stream (the 16 queues per set are FIFO).
  - Phase order: the two routed experts (weights streamed, read once), then
    the shared expert (weights resident in SBUF, read once).
"""

import numpy as np
import ml_dtypes

F8 = ml_dtypes.float8_e4m3   # matches mybir.dt.np(float8e4); NO saturation --
F16 = np.float16             # host must clip before casting to F8

# Model dims (hardcoded per problem spec nn_MoE_51616916963811)
D = 2048
INTER = 1408
E = 16
TOPK = 4
G = 4
TOPK_G = 2
T = 2048
SI = 2816           # shared inter dim
SI_SHARD = SI // 8  # 352
SI_PAD = 384        # padded to 3x128
ROUTE_SCALE = 1.0

NCORES = 8
ELOC = 2            # experts per core
TCHUNK = 512        # shared-expert token chunk
KD = D // 128       # 16 contraction chunks over D
KI = INTER // 128   # 11 tiles over INTER
KS = SI_PAD // 128  # 3 tiles over padded shared inter

A1 = 64.0           # host pre-scale on W1 (silu divides it back out)
A3 = 8.0            # host pre-scale on W3 (folded into gw epilogue)
A2 = 64.0           # host pre-scale on W2 (folded into gw epilogue)
F8CLIP = 440.0      # e4m3 overflows to inf past 448 -- clip before cast

CAP_MIN = 512       # capacity floor (expected count is exactly 512)
MAX_CAP = 704       # capacity ceiling (SBUF budget)

_CACHE = {}


def _roundup32(n):
    return ((int(n) + 31) // 32) * 32


def _chunks_of(cap):
    """Moving-dim chunks (PSUM banks are 512 fp32 wide)."""
    if cap <= 512:
        return (cap,)
    half = _roundup32((cap + 1) // 2)
    assert half <= 512
    return (half, cap - half)


def _pick_caps(counts):
    """Per-slot capacities from the sorted per-expert counts.

    Slot 0 holds the 8 largest experts (cap0 = max count), slot 1 the 8
    smallest (cap1 = 9th largest). Rounded to 32; clamped to
    [CAP_MIN, MAX_CAP] — kernel() truncates the (never expected) overflow
    tokens by lowest gate weight."""
    srt = np.sort(counts)[::-1]
    cap0 = max(CAP_MIN, min(MAX_CAP, _roundup32(srt[0])))
    cap1 = max(CAP_MIN, min(cap0, _roundup32(srt[NCORES])))
    return cap0, cap1


# ---------------------------------------------------------------- host gate --
def _route(x2d, Wg):
    """Replicates the reference gate in numpy float32.

    Returns topi [T, TOPK] int64 and weights [T, TOPK] float32."""
    logits = x2d.astype(np.float32) @ Wg.T.astype(np.float32)      # [T, E]
    m = logits.max(axis=-1, keepdims=True)
    ex = np.exp(logits - m)
    scores = ex / ex.sum(axis=-1, keepdims=True)                   # [T, E]
    sg = scores.reshape(T, G, E // G)
    gs = sg.max(axis=-1)                                           # [T, G]
    gidx = np.argsort(-gs, axis=1, kind="stable")[:, :TOPK_G]
    gmask = np.zeros((T, G), dtype=bool)
    np.put_along_axis(gmask, gidx, True, axis=1)
    masked = np.where(gmask[:, :, None], sg, -np.inf).reshape(T, E)
    topi = np.argsort(-masked, axis=1, kind="stable")[:, :TOPK]
    weights = np.take_along_axis(scores, topi, axis=1) * ROUTE_SCALE
    return topi, weights.astype(np.float32)


# ------------------------------------------------------------ host packing --
def _q8(a):
    return np.clip(a, -F8CLIP, F8CLIP).astype(F8)


def _tile_kxm4(w, scale):
    """[R, C] weight -> lhsT tiles [R/128, 128(p), C/128, 128] fp8 where
    tile[i, p, ko, m] = scale * w[i*128+m, ko*128+p]."""
    R, C = w.shape
    ri, ci = R // 128, C // 128
    return _q8(np.ascontiguousarray(
        (w * scale).reshape(ri, 128, ci, 128).transpose(0, 3, 2, 1)))


def _tile_kxm(w):
    """[R, C] weight -> lhsT tiles [R/128, 128(p), C/128 * 128] fp16."""
    R, C = w.shape
    ri, ci = R // 128, C // 128
    return np.ascontiguousarray(
        w.reshape(ri, 128, ci, 128).transpose(0, 3, 2, 1)
    ).reshape(ri, 128, ci * 128).astype(F16)


def _tile_xT(xrows, cap):
    """[n, D] activations -> [128(p), KD, cap] fp8, zero-padded to cap."""
    n = xrows.shape[0]
    out = np.zeros((128, KD, cap), dtype=np.float32)
    xt = xrows.T.reshape(KD, 128, n).transpose(1, 0, 2)  # [128, KD, n]
    out[:, :, :n] = xt
    return _q8(out)


# ------------------------------------------------------------- bass kernel --
def _build_nc(caps):
    import concourse.bass as bass
    import concourse.tile as tile
    from concourse import bacc, mybir

    f32 = mybir.dt.float32
    f16 = mybir.dt.float16
    f8 = mybir.dt.float8e4
    DR = mybir.MatmulPerfMode.DoubleRow
    AF = mybir.ActivationFunctionType

    cap0, cap1 = caps
    CHK = [_chunks_of(cap0), _chunks_of(cap1)]
    OFF = []
    for chunks in CHK:
        off, lst = 0, []
        for w in chunks:
            lst.append((off, w))
            off += w
        OFF.append(lst)
    MAXW = max(max(c) for c in CHK)
    NCT = T // TCHUNK
    KP = KD // 2        # 8 contraction k-tile pairs over D
    IP = KI // 2        # 5 full pairs over INTER (plus one odd tile)

    nc = bacc.Bacc("TRN2", target_bir_lowering=False, debug=False,
                   enable_asserts=False)

    # Inputs (per core). Routed operands fp8 (DoubleRow), shared fp16.
    xg = nc.dram_tensor("xg", [ELOC, 128, KD, cap0], f8, kind="ExternalInput").ap()
    gw = nc.dram_tensor("gw", [ELOC, 128, cap0], f32, kind="ExternalInput").ap()
    w1 = nc.dram_tensor("w1", [ELOC, KI, 128, KD, 128], f8, kind="ExternalInput").ap()
    w3 = nc.dram_tensor("w3", [ELOC, KI, 128, KD, 128], f8, kind="ExternalInput").ap()
    w2 = nc.dram_tensor("w2", [ELOC, KD, 128, KI, 128], f8, kind="ExternalInput").ap()
    xt = nc.dram_tensor("xt", [T // TCHUNK, 128, KD, TCHUNK], f16, kind="ExternalInput").ap()
    ws1 = nc.dram_tensor("ws1", [KS, 128, KD * 128], f16, kind="ExternalInput").ap()
    ws3 = nc.dram_tensor("ws3", [KS, 128, KD * 128], f16, kind="ExternalInput").ap()
    ws2 = nc.dram_tensor("ws2", [KD, 128, KS * 128], f16, kind="ExternalInput").ap()
    # Outputs (layouts chosen so stores batch into few huge DMAs: the
    # in-order issuing engines pay ~600ns per DMA instruction, so frequent
    # small stores would pace the PSUM-drain path)
    yt = nc.dram_tensor("yt", [ELOC, 128, KD, cap0], f16, kind="ExternalOutput").ap()
    zt = nc.dram_tensor("zt", [T // TCHUNK, 2, 128, KD // 2, TCHUNK], f16,
                        kind="ExternalOutput").ap()

    with tile.TileContext(nc) as tc:
        # All pools coexist (~140KB of 208KB SBUF) so resident tiles never
        # WAR-wait on streamed ones: a dma_start with an unmet wait stalls
        # the whole in-order SP issue stream behind it.
        wg12 = tc.alloc_tile_pool(name="wg12", bufs=3)
        xs = tc.alloc_tile_pool(name="xs", bufs=3)
        pg12 = tc.alloc_tile_pool(name="pg12", bufs=2, space="PSUM")
        pg3 = tc.alloc_tile_pool(name="pg3", bufs=4, space="PSUM")
        shres = tc.alloc_tile_pool(name="shres", bufs=1)   # shared residents
        wg3 = tc.alloc_tile_pool(name="wg3", bufs=8)
        htp = tc.alloc_tile_pool(name="htp", bufs=1)
        gwp = tc.alloc_tile_pool(name="gwp", bufs=2)
        actp = tc.alloc_tile_pool(name="actp", bufs=3)
        stg = tc.alloc_tile_pool(name="stg", bufs=2)
        hstp = tc.alloc_tile_pool(name="hstp", bufs=2)
        acts = tc.alloc_tile_pool(name="acts", bufs=3)
        stgs = tc.alloc_tile_pool(name="stgs", bufs=2)

        ws1r = [None] * KS
        ws3r = [None] * KS
        ws2r = [None, None]
        xt_tiles = {}

        # ---------------- routed experts, weights streamed ----------------
        for s in range(ELOC):
            cap = caps[s]
            coff = OFF[s]
            if s == 0:
                # startup order: first i-tile weights, then tokens in
                # ko-quarters (contiguous lines; matmul 0 starts after the
                # first quarter)
                w1t0 = wg12.tile([128, KD, 128], f8, tag="w1t", name="w1t0_0")
                nc.sync.dma_start(w1t0[:], w1[0, 0])
                xg_s = xs.tile([128, KD, cap0], f8, tag="x", name="xg0")
                for kq in range(0, KD, 4):
                    nc.sync.dma_start(xg_s[:, kq:kq + 4, :cap],
                                      xg[0, :, kq:kq + 4, :cap])
                w3t0 = wg12.tile([128, KD, 128], f8, tag="w3t", name="w3t0_0")
                nc.sync.dma_start(w3t0[:], w3[0, 0])
                gw_s = gwp.tile([128, cap0], f32, tag="gw", name="gw0")
                nc.sync.dma_start(gw_s[:, :cap], gw[0, :, :cap])
            else:
                xg_s, gw_s = xg_next, gw_next

            ht = htp.tile([128, KI, cap0], f8, tag="ht", name=f"ht{s}")

            # GEMM1/2 (DoubleRow): hT[i, c] = silu(x @ W1^T) * (x @ W3^T)
            for i in range(KI):
                if s == 0 and i == 0:
                    w1t, w3t = w1t0, w3t0
                else:
                    w1t = wg12.tile([128, KD, 128], f8, tag="w1t", name=f"w1t{s}_{i}")
                    nc.sync.dma_start(w1t[:], w1[s, i])
                    w3t = wg12.tile([128, KD, 128], f8, tag="w3t", name=f"w3t{s}_{i}")
                    nc.sync.dma_start(w3t[:], w3[s, i])
                for (c0, cw) in coff:
                    p1 = pg12.tile([128, MAXW], f32, tag="p1", name="p1")
                    p3 = pg12.tile([128, MAXW], f32, tag="p3", name="p3")
                    for kp in range(KP):
                        nc.tensor.matmul(
                            p1[:, :cw], w1t[:, 2 * kp:2 * kp + 2, :],
                            xg_s[:, 2 * kp:2 * kp + 2, c0:c0 + cw],
                            start=(kp == 0), stop=(kp == KP - 1), perf_mode=DR)
                    for kp in range(KP):
                        nc.tensor.matmul(
                            p3[:, :cw], w3t[:, 2 * kp:2 * kp + 2, :],
                            xg_s[:, 2 * kp:2 * kp + 2, c0:c0 + cw],
                            start=(kp == 0), stop=(kp == KP - 1), perf_mode=DR)
                    a1 = actp.tile([128, MAXW], f16, tag="act", name="a1")
                    nc.scalar.activation(a1[:, :cw], p1[:, :cw], AF.Silu,
                                         scale=1.0 / A1)
                    nc.vector.tensor_mul(ht[:, i, c0:c0 + cw], a1[:, :cw],
                                         p3[:, :cw])
                # next expert's tokens prefetch rides s0-G12's SP slack in
                # quarter-size pieces so the w1/w3 stream is never displaced
                if s == 0 and i in (6, 7, 8, 9):
                    kq = 4 * (i - 6)
                    if i == 6:
                        xg_next = xs.tile([128, KD, cap0], f8, tag="x", name="xg1")
                    nc.sync.dma_start(xg_next[:, kq:kq + 4, :caps[1]],
                                      xg[1, :, kq:kq + 4, :caps[1]])
                    if i == 9:
                        gw_next = gwp.tile([128, cap0], f32, tag="gw", name="gw1")
                        nc.sync.dma_start(gw_next[:, :caps[1]], gw[1, :, :caps[1]])
                # shared-expert residents + first token chunks stream on the
                # Act queue during s1-G12 (it is empty there), touching
                # neither the SP weight stream nor the G3 store path
                if s == 1 and i < 6:
                    j, which = divmod(i, 2)
                    if which == 0:
                        ws1r[j] = shres.tile([128, KD * 128], f16, tag=f"ws1r{j}",
                                             name=f"ws1r{j}")
                        nc.scalar.dma_start(ws1r[j][:], ws1[j])
                    else:
                        ws3r[j] = shres.tile([128, KD * 128], f16, tag=f"ws3r{j}",
                                             name=f"ws3r{j}")
                        nc.scalar.dma_start(ws3r[j][:], ws3[j])
                if s == 1 and i in (6, 7):
                    j = i - 6
                    ws2r[j] = shres.tile([128, KD // 2, KS * 128], f16,
                                         tag=f"ws2r{j}", name=f"ws2r{j}")
                    for dd in range(KD // 2):
                        nc.scalar.dma_start(ws2r[j][:, dd], ws2[j * (KD // 2) + dd])
                if s == 1 and i in (8, 9):
                    ct0 = i - 8
                    xt_tiles[ct0] = xs.tile([128, KD, TCHUNK], f16, tag="x",
                                            name=f"xt{ct0}")
                    nc.scalar.dma_start(xt_tiles[ct0][:], xt[ct0])

            # GEMM3 (DoubleRow + one odd tile): yT = (hT^T @ W2^T)^T * gate.
            # Epilogue products collect in one staging tile; two big DMAs
            # per expert instead of 2 per d keep the issue engines off the
            # critical path.
            ytst = stg.tile([128, KD, cap0], f16, tag="st", name=f"ytst{s}")
            for d in range(KD):
                w2t = wg3.tile([128, KI, 128], f8, tag="w2t", name=f"w2t{s}_{d}")
                nc.sync.dma_start(w2t[:], w2[s, d])
                for (c0, cw) in coff:
                    py = pg3.tile([128, MAXW], f32, tag="py", name="py")
                    for ip in range(IP):
                        nc.tensor.matmul(
                            py[:, :cw], w2t[:, 2 * ip:2 * ip + 2, :],
                            ht[:, 2 * ip:2 * ip + 2, c0:c0 + cw],
                            start=(ip == 0), stop=False, perf_mode=DR)
                    nc.tensor.matmul(
                        py[:, :cw], w2t[:, KI - 1, :],
                        ht[:, KI - 1, c0:c0 + cw],
                        start=False, stop=True)
                    nc.vector.tensor_mul(ytst[:, d, c0:c0 + cw], py[:, :cw],
                                         gw_s[:, c0:c0 + cw])
                if d == KD // 2 - 1:
                    nc.scalar.dma_start(yt[s, :, :KD // 2, :cap],
                                        ytst[:, :KD // 2, :cap])
                if d == KD - 1:
                    nc.scalar.dma_start(yt[s, :, KD // 2:, :cap],
                                        ytst[:, KD // 2:, :cap])

        # -------- shared expert (inter-sharded, fp16), weights resident ----
        # software pipeline: G1(ct) then G2(ct-1), so the PSUM drain /
        # vector-mul tail of hst(ct) hides behind the next chunk's matmuls
        def g1(ct):
            # SP is idle in the shared phase: xt loads and zt stores go
            # there, leaving the scalar engine free for silu + PSUM drains
            if ct + 2 < NCT:
                xt_tiles[ct + 2] = xs.tile([128, KD, TCHUNK], f16, tag="x",
                                           name=f"xt{ct + 2}")
                nc.sync.dma_start(xt_tiles[ct + 2][:], xt[ct + 2])
            xt_c = xt_tiles.pop(ct)
            hst = hstp.tile([128, KS, TCHUNK], f16, tag="hst", name=f"hst{ct}")
            for i in range(KS):
                p1 = pg12.tile([128, TCHUNK], f32, tag="p1", name="p1")
                p3 = pg12.tile([128, TCHUNK], f32, tag="p3", name="p3")
                for ko in range(KD):
                    nc.tensor.matmul(
                        p1[:], ws1r[i][:, ko * 128:(ko + 1) * 128],
                        xt_c[:, ko, :],
                        start=(ko == 0), stop=(ko == KD - 1))
                for ko in range(KD):
                    nc.tensor.matmul(
                        p3[:], ws3r[i][:, ko * 128:(ko + 1) * 128],
                        xt_c[:, ko, :],
                        start=(ko == 0), stop=(ko == KD - 1))
                a1 = acts.tile([128, TCHUNK], f16, tag="acts", name="a1")
                nc.scalar.activation(a1[:], p1[:], AF.Silu)
                nc.vector.tensor_mul(hst[:, i], a1[:], p3[:])
            return hst

        def g2(ct, hst):
            for half in range(2):
                ztst = stgs.tile([128, KD // 2, TCHUNK], f16, tag="stgs",
                                 name=f"ztst{ct}_{half}")
                for dd in range(KD // 2):
                    d = half * (KD // 2) + dd
                    py = pg3.tile([128, TCHUNK], f32, tag="py", name="py")
                    w2half = ws2r[half]
                    for io in range(KS):
                        nc.tensor.matmul(
                            py[:], w2half[:, dd, io * 128:(io + 1) * 128],
                            hst[:, io],
                            start=(io == 0), stop=(io == KS - 1))
                    # alternate the PSUM->SBUF drain across two engines so
                    # it keeps up with the 3-matmul fill
                    if d % 2 == 0:
                        nc.scalar.activation(ztst[:, dd], py[:], AF.Copy)
                    else:
                        nc.vector.tensor_copy(ztst[:, dd], py[:])
                nc.sync.dma_start(zt[ct, half], ztst[:])

        hst_prev = None
        for ct in range(NCT):
            hst = g1(ct)
            if hst_prev is not None:
                g2(ct - 1, hst_prev)
            hst_prev = hst
        g2(NCT - 1, hst_prev)

        stgs.release()
        acts.release()
        hstp.release()
        stg.release()
        actp.release()
        gwp.release()
        htp.release()
        wg3.release()
        shres.release()
        pg3.release()
        pg12.release()
        xs.release()
        wg12.release()

    nc.compile()
    return nc


def _get_nc(caps):
    key = ("nc", caps)
    if key not in _CACHE:
        _CACHE[key] = _build_nc(caps)
    return _CACHE[key]


# ------------------------------------------------------------------ kernel --
def kernel(x, Wg, W1, W3, W2, Ws1, Ws3, Ws2):
    from concourse.bass_utils import run_bass_kernel_spmd

    x = np.asarray(x, dtype=np.float32)
    x2d = np.ascontiguousarray(x.reshape(T, D))
    Wg = np.asarray(Wg, dtype=np.float32)
    W1 = np.asarray(W1, dtype=np.float32)
    W3 = np.asarray(W3, dtype=np.float32)
    W2 = np.asarray(W2, dtype=np.float32)
    Ws1 = np.asarray(Ws1, dtype=np.float32)
    Ws3 = np.asarray(Ws3, dtype=np.float32)
    Ws2 = np.asarray(Ws2, dtype=np.float32)

    # ---- host routing + dispatch ----
    topi, weights = _route(x2d, Wg)
    flat_e = topi.ravel()
    flat_t = np.repeat(np.arange(T), TOPK)
    flat_w = weights.ravel()
    order = np.argsort(flat_e, kind="stable")
    se, st_, sw = flat_e[order], flat_t[order], flat_w[order]
    bounds = np.searchsorted(se, np.arange(E + 1))
    tok_of = [st_[bounds[e]:bounds[e + 1]] for e in range(E)]
    wt_of = [sw[bounds[e]:bounds[e + 1]] for e in range(E)]
    counts = np.array([len(t) for t in tok_of])

    caps = _pick_caps(counts)
    cap0, cap1 = caps
    # expert -> (core, slot): slot 0 gets the 8 largest, slot 1 the 8
    # smallest (largest paired with smallest so per-core work is even)
    ranks = np.argsort(-counts, kind="stable")
    assign = [[int(ranks[c]), int(ranks[2 * NCORES - 1 - c])]
              for c in range(NCORES)]

    truncated = False
    for e in range(E):
        cap_e = cap0 if any(a[0] == e for a in assign) else cap1
        if len(tok_of[e]) > cap_e:  # only possible at the MAX_CAP clamp
            keep = np.sort(np.argsort(-wt_of[e], kind="stable")[:cap_e])
            tok_of[e] = tok_of[e][keep]
            wt_of[e] = wt_of[e][keep]
            truncated = True

    # ---- build per-core input maps ----
    xt_full = x2d.T.reshape(KD, 128, T).transpose(1, 0, 2)  # [128, KD, T]
    xt_tiles = np.ascontiguousarray(
        np.stack([xt_full[:, :, i * TCHUNK:(i + 1) * TCHUNK]
                  for i in range(T // TCHUNK)])).astype(F16)

    in_maps = []
    for c in range(NCORES):
        exps = assign[c]
        xg_c = np.stack([_tile_xT(x2d[tok_of[e]], cap0) for e in exps])
        # epilogue scale: py = (A2*W2)@(A3*h) => multiply by gw/(A2*A3)
        gw_c = np.zeros((ELOC, 128, cap0), dtype=np.float32)
        for s, e in enumerate(exps):
            gw_c[s, :, :len(wt_of[e])] = wt_of[e][None, :] / (A2 * A3)
        w1_c = np.stack([_tile_kxm4(W1[e], A1) for e in exps])
        w3_c = np.stack([_tile_kxm4(W3[e], A3) for e in exps])
        w2_c = np.stack([_tile_kxm4(W2[e], A2) for e in exps])

        lo = c * SI_SHARD
        ws1_s = np.zeros((SI_PAD, D), dtype=np.float32)
        ws1_s[:SI_SHARD] = Ws1[lo:lo + SI_SHARD]
        ws3_s = np.zeros((SI_PAD, D), dtype=np.float32)
        ws3_s[:SI_SHARD] = Ws3[lo:lo + SI_SHARD]
        ws2_s = np.zeros((D, SI_PAD), dtype=np.float32)
        ws2_s[:, :SI_SHARD] = Ws2[:, lo:lo + SI_SHARD]

        in_maps.append({
            "xg": xg_c, "gw": gw_c, "w1": w1_c, "w3": w3_c, "w2": w2_c,
            "xt": xt_tiles,
            "ws1": _tile_kxm(ws1_s), "ws3": _tile_kxm(ws3_s),
            "ws2": _tile_kxm(ws2_s),
        })

    # ---- run on 8 cores ----
    shapes = {
        "xg": ((ELOC, 128, KD, cap0), F8), "gw": ((ELOC, 128, cap0), np.float32),
        "w1": ((ELOC, KI, 128, KD, 128), F8), "w3": ((ELOC, KI, 128, KD, 128), F8),
        "w2": ((ELOC, KD, 128, KI, 128), F8),
        "xt": ((T // TCHUNK, 128, KD, TCHUNK), F16),
        "ws1": ((KS, 128, KD * 128), F16), "ws3": ((KS, 128, KD * 128), F16),
        "ws2": ((KD, 128, KS * 128), F16),
    }
    for m in in_maps:
        for k, v in m.items():
            assert v.shape == shapes[k][0], (k, v.shape, shapes[k][0])
            assert v.dtype == shapes[k][1], (k, v.dtype)

    nc = _get_nc(caps)
    res = run_bass_kernel_spmd(nc, in_maps, core_ids=list(range(NCORES)))
    _CACHE["last_results"] = res

    # ---- combine on host ----
    # routed: yt[s, d, p, c] = w * Y[c, d*128+p]
    cat_tok = []
    cat_rows = []
    for c in range(NCORES):
        ytc = res.results[c]["yt"]  # [ELOC, 128, KD, cap0] fp16
        for s in range(ELOC):
            e = assign[c][s]
            n = len(tok_of[e])
            rows = ytc[s].transpose(2, 1, 0).reshape(cap0, D)[:n].astype(
                np.float32)  # [n, D]
            cat_tok.append(tok_of[e])
            cat_rows.append(rows)
    cat_tok = np.concatenate(cat_tok)
    cat_rows = np.concatenate(cat_rows, axis=0)
    if truncated:
        y = np.zeros((T, D), dtype=np.float32)
        np.add.at(y, cat_tok, cat_rows)
    else:
        # every token has exactly TOPK contributions
        order = np.argsort(cat_tok, kind="stable")
        y = cat_rows[order].reshape(T, TOPK, D).sum(axis=1)

    # shared: sum partials; zt[ct, half, p, dd, t'] = Z[t, d*128+p] with
    # d = half*8+dd, t = ct*TCHUNK+t'
    z_acc = res.results[0]["zt"].astype(np.float32)
    for c in range(1, NCORES):
        z_acc += res.results[c]["zt"].astype(np.float32)
    z = z_acc.transpose(1, 3, 2, 0, 4).reshape(D, T).T  # [T, D]

    return (y + z).reshape(1, T, D).astype(np.float32)


# revision 34
# speedup vs baseline: 1.1401x; 1.1369x over previous
"""MoE (group-limited top-k routing) Trainium2 kernel, expert-parallel on 8 cores.

Strategy:
  - Host (numpy): gate softmax + group-limited top-4 routing (control plane,
    ~0.06% of FLOPs), token dispatch (gather per expert) and final combine.
  - Device (8 NeuronCores, SPMD): every core owns two routed experts — slot 0
    gets one of the 8 most-loaded experts (capacity cap0), slot 1 one of the 8
    least-loaded (cap1 <= cap0) — so padding tracks the real count
    distribution instead of the global max.
  - Routed experts run in fp8 e4m3 with DoubleRow perf mode (2 contraction
    k-tiles per matmul at 0.5 cycles/row). Weights are pre-scaled on host
    (W1*64, W3*8, W2*64) to dodge e4m3's tiny subnormal range; the 1/64 comes
    out inside the silu (activation scale) and the rest (1/512) is folded
    into the gate-weight epilogue. h is stored fp8 (= 8*h_true, max ~104,
    well inside e4m3's +-448). Routed error is diluted ~5x because the
    shared expert carries ~98% of the output norm; measured total rel err
    ~1.4e-2 against the 2e-2 gate.
  - The shared expert (carrying the norm) runs in fp16 (same PE rate as
    bf16, 8x finer mantissa), inter-dim sharded (2816/8=352, padded to 384);
    each core computes a partial z for all 2048 tokens; host sums partials.
  - Weight/token loads stream on the SP HWDGE queue set; output stores and
    shared-token loads go on the Activation HWDGE set so stores can never
    head-block the weight stream (the 16 queues per set are FIFO).
  - Phase order: the two routed experts (weights streamed, read once), then
    the shared expert (weights resident in SBUF, read once).
"""

import numpy as np
import ml_dtypes

F8 = ml_dtypes.float8_e4m3   # matches mybir.dt.np(float8e4); NO saturation --
F16 = np.float16             # host must clip before casting to F8

# Model dims (hardcoded per problem spec nn_MoE_51616916963811)
D = 2048
INTER = 1408
E = 16
TOPK = 4
G = 4
TOPK_G = 2
T = 2048
SI = 2816           # shared inter dim
SI_SHARD = SI // 8  # 352
SI_PAD = 384        # padded to 3x128
ROUTE_SCALE = 1.0

NCORES = 8
ELOC = 2            # experts per core
TCHUNK = 512        # shared-expert token chunk
KD = D // 128       # 16 contraction chunks over D
KI = INTER // 128   # 11 tiles over INTER
KS = SI_PAD // 128  # 3 tiles over padded shared inter

A1 = 64.0           # host pre-scale on W1 (silu divides it back out)
A3 = 8.0            # host pre-scale on W3 (folded into gw epilogue)
A2 = 64.0           # host pre-scale on W2 (folded into gw epilogue)
F8CLIP = 440.0      # e4m3 overflows to inf past 448 -- clip before cast

CAP_MIN = 512       # capacity floor (expected count is exactly 512)
MAX_CAP = 704       # capacity ceiling (SBUF budget)

_CACHE = {}


def _roundup32(n):
    return ((int(n) + 31) // 32) * 32


def _chunks_of(cap):
    """Moving-dim chunks (PSUM banks are 512 fp32 wide)."""
    if cap <= 512:
        return (cap,)
    half = _roundup32((cap + 1) // 2)
    assert half <= 512
    return (half, cap - half)


def _pick_caps(counts):
    """Per-slot capacities from the sorted per-expert counts.

    Slot 0 holds the 8 largest experts (cap0 = max count), slot 1 the 8
    smallest (cap1 = 9th largest). Rounded to 32; clamped to
    [CAP_MIN, MAX_CAP] — kernel() truncates the (never expected) overflow
    tokens by lowest gate weight."""
    srt = np.sort(counts)[::-1]
    cap0 = max(CAP_MIN, min(MAX_CAP, _roundup32(srt[0])))
    cap1 = max(CAP_MIN, min(cap0, _roundup32(srt[NCORES])))
    return cap0, cap1


# ---------------------------------------------------------------- host gate --
def _route(x2d, Wg):
    """Replicates the reference gate in numpy float32.

    Returns topi [T, TOPK] int64 and weights [T, TOPK] float32."""
    logits = x2d.astype(np.float32) @ Wg.T.astype(np.float32)      # [T, E]
    m = logits.max(axis=-1, keepdims=True)
    ex = np.exp(logits - m)
    scores = ex / ex.sum(axis=-1, keepdims=True)                   # [T, E]
    sg = scores.reshape(T, G, E // G)
    gs = sg.max(axis=-1)                                           # [T, G]
    gidx = np.argsort(-gs, axis=1, kind="stable")[:, :TOPK_G]
    gmask = np.zeros((T, G), dtype=bool)
    np.put_along_axis(gmask, gidx, True, axis=1)
    masked = np.where(gmask[:, :, None], sg, -np.inf).reshape(T, E)
    topi = np.argsort(-masked, axis=1, kind="stable")[:, :TOPK]
    weights = np.take_along_axis(scores, topi, axis=1) * ROUTE_SCALE
    return topi, weights.astype(np.float32)


# ------------------------------------------------------------ host packing --
def _q8(a):
    return np.clip(a, -F8CLIP, F8CLIP).astype(F8)


def _tile_kxm4(w, scale):
    """[R, C] weight -> lhsT tiles [R/128, 128(p), C/128, 128] fp8 where
    tile[i, p, ko, m] = scale * w[i*128+m, ko*128+p]."""
    R, C = w.shape
    ri, ci = R // 128, C // 128
    return _q8(np.ascontiguousarray(
        (w * scale).reshape(ri, 128, ci, 128).transpose(0, 3, 2, 1)))


def _tile_kxm(w):
    """[R, C] weight -> lhsT tiles [R/128, 128(p), C/128 * 128] fp16."""
    R, C = w.shape
    ri, ci = R // 128, C // 128
    return np.ascontiguousarray(
        w.reshape(ri, 128, ci, 128).transpose(0, 3, 2, 1)
    ).reshape(ri, 128, ci * 128).astype(F16)


def _tile_xT(xrows, cap):
    """[n, D] activations -> [128(p), KD, cap] fp8, zero-padded to cap."""
    n = xrows.shape[0]
    out = np.zeros((128, KD, cap), dtype=np.float32)
    xt = xrows.T.reshape(KD, 128, n).transpose(1, 0, 2)  # [128, KD, n]
    out[:, :, :n] = xt
    return _q8(out)


# ------------------------------------------------------------- bass kernel --
def _build_nc(caps):
    import concourse.bass as bass
    import concourse.tile as tile
    from concourse import bacc, mybir

    f32 = mybir.dt.float32
    f16 = mybir.dt.float16
    f8 = mybir.dt.float8e4
    DR = mybir.MatmulPerfMode.DoubleRow
    AF = mybir.ActivationFunctionType

    cap0, cap1 = caps
    CHK = [_chunks_of(cap0), _chunks_of(cap1)]
    OFF = []
    for chunks in CHK:
        off, lst = 0, []
        for w in chunks:
            lst.append((off, w))
            off += w
        OFF.append(lst)
    MAXW = max(max(c) for c in CHK)
    NCT = T // TCHUNK
    KP = KD // 2        # 8 contraction k-tile pairs over D
    IP = KI // 2        # 5 full pairs over INTER (plus one odd tile)

    nc = bacc.Bacc("TRN2", target_bir_lowering=False, debug=False,
                   enable_asserts=False)

    # Inputs (per core). Routed operands fp8 (DoubleRow), shared fp16.
    xg = nc.dram_tensor("xg", [ELOC, 128, KD, cap0], f8, kind="ExternalInput").ap()
    gw = nc.dram_tensor("gw", [ELOC, 128, cap0], f32, kind="ExternalInput").ap()
    w1 = nc.dram_tensor("w1", [ELOC, KI, 128, KD, 128], f8, kind="ExternalInput").ap()
    w3 = nc.dram_tensor("w3", [ELOC, KI, 128, KD, 128], f8, kind="ExternalInput").ap()
    w2 = nc.dram_tensor("w2", [ELOC, KD, 128, KI, 128], f8, kind="ExternalInput").ap()
    xt = nc.dram_tensor("xt", [T // TCHUNK, 128, KD, TCHUNK], f16, kind="ExternalInput").ap()
    ws1 = nc.dram_tensor("ws1", [KS, 128, KD * 128], f16, kind="ExternalInput").ap()
    ws3 = nc.dram_tensor("ws3", [KS, 128, KD * 128], f16, kind="ExternalInput").ap()
    ws2 = nc.dram_tensor("ws2", [KD, 128, KS * 128], f16, kind="ExternalInput").ap()
    # Outputs (layouts chosen so stores batch into few huge DMAs: the
    # in-order issuing engines pay ~600ns per DMA instruction, so frequent
    # small stores would pace the PSUM-drain path)
    yt = nc.dram_tensor("yt", [ELOC, 128, KD, cap0], f16, kind="ExternalOutput").ap()
    zt = nc.dram_tensor("zt", [T // TCHUNK, 2, 128, KD // 2, TCHUNK], f16,
                        kind="ExternalOutput").ap()

    with tile.TileContext(nc) as tc:
        # All pools coexist (~140KB of 208KB SBUF) so resident tiles never
        # WAR-wait on streamed ones: a dma_start with an unmet wait stalls
        # the whole in-order SP issue stream behind it.
        wg12 = tc.alloc_tile_pool(name="wg12", bufs=3)
        xs = tc.alloc_tile_pool(name="xs", bufs=3)
        pg12 = tc.alloc_tile_pool(name="pg12", bufs=2, space="PSUM")
        pg3 = tc.alloc_tile_pool(name="pg3", bufs=4, space="PSUM")
        shres = tc.alloc_tile_pool(name="shres", bufs=1)   # shared residents
        wg3 = tc.alloc_tile_pool(name="wg3", bufs=5)
        htp = tc.alloc_tile_pool(name="htp", bufs=1)
        gwp = tc.alloc_tile_pool(name="gwp", bufs=2)
        actp = tc.alloc_tile_pool(name="actp", bufs=3)
        stg = tc.alloc_tile_pool(name="stg", bufs=2)
        hstp = tc.alloc_tile_pool(name="hstp", bufs=2)
        acts = tc.alloc_tile_pool(name="acts", bufs=3)
        stgs = tc.alloc_tile_pool(name="stgs", bufs=2)

        ws1r = [None] * KS
        ws3r = [None] * KS
        ws2r = [None, None]
        xt_tiles = {}

        # ---------------- routed experts, weights streamed ----------------
        for s in range(ELOC):
            cap = caps[s]
            coff = OFF[s]
            if s == 0:
                # startup order: first i-tile weights, then tokens in
                # ko-quarters (contiguous lines; matmul 0 starts after the
                # first quarter)
                w1t0 = wg12.tile([128, KD, 128], f8, tag="w1t", name="w1t0_0")
                nc.sync.dma_start(w1t0[:], w1[0, 0])
                xg_s = xs.tile([128, KD, cap0], f8, tag="x", name="xg0")
                nc.sync.dma_start(xg_s[:, 0:4, :cap], xg[0, :, 0:4, :cap])
                # w3t0 ahead of the remaining token quarters: the GEMM2 loop
                # starts ~1us in and must not wait behind 0.75MB of tokens
                w3t0 = wg12.tile([128, KD, 128], f8, tag="w3t", name="w3t0_0")
                nc.sync.dma_start(w3t0[:], w3[0, 0])
                for kq in range(4, KD, 4):
                    nc.sync.dma_start(xg_s[:, kq:kq + 4, :cap],
                                      xg[0, :, kq:kq + 4, :cap])
                gw_s = gwp.tile([128, cap0], f32, tag="gw", name="gw0")
                nc.sync.dma_start(gw_s[:, :cap], gw[0, :, :cap])
            else:
                xg_s, gw_s = xg_next, gw_next

            ht = htp.tile([128, KI, cap0], f8, tag="ht", name=f"ht{s}")

            # GEMM1/2 (DoubleRow): hT[i, c] = silu(x @ W1^T) * (x @ W3^T)
            for i in range(KI):
                if s == 0 and i == 0:
                    w1t, w3t = w1t0, w3t0
                else:
                    w1t = wg12.tile([128, KD, 128], f8, tag="w1t", name=f"w1t{s}_{i}")
                    nc.sync.dma_start(w1t[:], w1[s, i])
                    w3t = wg12.tile([128, KD, 128], f8, tag="w3t", name=f"w3t{s}_{i}")
                    nc.sync.dma_start(w3t[:], w3[s, i])
                for (c0, cw) in coff:
                    p1 = pg12.tile([128, MAXW], f32, tag="p1", name="p1")
                    p3 = pg12.tile([128, MAXW], f32, tag="p3", name="p3")
                    for kp in range(KP):
                        nc.tensor.matmul(
                            p1[:, :cw], w1t[:, 2 * kp:2 * kp + 2, :],
                            xg_s[:, 2 * kp:2 * kp + 2, c0:c0 + cw],
                            start=(kp == 0), stop=(kp == KP - 1), perf_mode=DR)
                    for kp in range(KP):
                        nc.tensor.matmul(
                            p3[:, :cw], w3t[:, 2 * kp:2 * kp + 2, :],
                            xg_s[:, 2 * kp:2 * kp + 2, c0:c0 + cw],
                            start=(kp == 0), stop=(kp == KP - 1), perf_mode=DR)
                    a1 = actp.tile([128, MAXW], f16, tag="act", name="a1")
                    nc.scalar.activation(a1[:, :cw], p1[:, :cw], AF.Silu,
                                         scale=1.0 / A1)
                    nc.vector.tensor_mul(ht[:, i, c0:c0 + cw], a1[:, :cw],
                                         p3[:, :cw])
                # next expert's tokens prefetch rides s0-G12's SP slack in
                # quarter-size pieces so the w1/w3 stream is never displaced
                if s == 0 and i in (6, 7, 8, 9):
                    kq = 4 * (i - 6)
                    if i == 6:
                        xg_next = xs.tile([128, KD, cap0], f8, tag="x", name="xg1")
                    nc.sync.dma_start(xg_next[:, kq:kq + 4, :caps[1]],
                                      xg[1, :, kq:kq + 4, :caps[1]])
                    if i == 9:
                        gw_next = gwp.tile([128, cap0], f32, tag="gw", name="gw1")
                        nc.sync.dma_start(gw_next[:, :caps[1]], gw[1, :, :caps[1]])
                # shared-expert residents + first token chunks stream on the
                # Act queue during s1-G12 (it is empty there), touching
                # neither the SP weight stream nor the G3 store path
                if s == 1 and i < 6:
                    j, which = divmod(i, 2)
                    if which == 0:
                        ws1r[j] = shres.tile([128, KD * 128], f16, tag=f"ws1r{j}",
                                             name=f"ws1r{j}")
                        nc.scalar.dma_start(ws1r[j][:], ws1[j])
                    else:
                        ws3r[j] = shres.tile([128, KD * 128], f16, tag=f"ws3r{j}",
                                             name=f"ws3r{j}")
                        nc.scalar.dma_start(ws3r[j][:], ws3[j])
                if s == 1 and i in (6, 7):
                    j = i - 6
                    ws2r[j] = shres.tile([128, KD // 2, KS * 128], f16,
                                         tag=f"ws2r{j}", name=f"ws2r{j}")
                    for dd in range(KD // 2):
                        nc.scalar.dma_start(ws2r[j][:, dd], ws2[j * (KD // 2) + dd])
                if s == 1 and i in (8, 9):
                    ct0 = i - 8
                    xt_tiles[ct0] = xs.tile([128, KD, TCHUNK], f16, tag="x",
                                            name=f"xt{ct0}")
                    nc.scalar.dma_start(xt_tiles[ct0][:], xt[ct0])

            # GEMM3 (DoubleRow + one odd tile): yT = (hT^T @ W2^T)^T * gate.
            # Epilogue products collect in one staging tile; two big DMAs
            # per expert instead of 2 per d keep the issue engines off the
            # critical path.
            ytst = stg.tile([128, KD, cap0], f16, tag="st", name=f"ytst{s}")
            for d in range(KD):
                w2t = wg3.tile([128, KI, 128], f8, tag="w2t", name=f"w2t{s}_{d}")
                nc.sync.dma_start(w2t[:], w2[s, d])
                for (c0, cw) in coff:
                    py = pg3.tile([128, MAXW], f32, tag="py", name="py")
                    for ip in range(IP):
                        nc.tensor.matmul(
                            py[:, :cw], w2t[:, 2 * ip:2 * ip + 2, :],
                            ht[:, 2 * ip:2 * ip + 2, c0:c0 + cw],
                            start=(ip == 0), stop=False, perf_mode=DR)
                    nc.tensor.matmul(
                        py[:, :cw], w2t[:, KI - 1, :],
                        ht[:, KI - 1, c0:c0 + cw],
                        start=False, stop=True)
                    nc.vector.tensor_mul(ytst[:, d, c0:c0 + cw], py[:, :cw],
                                         gw_s[:, c0:c0 + cw])
                if d == KD // 2 - 1:
                    nc.scalar.dma_start(yt[s, :, :KD // 2, :cap],
                                        ytst[:, :KD // 2, :cap])
                if d == KD - 1:
                    nc.scalar.dma_start(yt[s, :, KD // 2:, :cap],
                                        ytst[:, KD // 2:, :cap])

        # -------- shared expert (inter-sharded, fp16), weights resident ----
        # software pipeline: G1(ct) then G2(ct-1), so the PSUM drain /
        # vector-mul tail of hst(ct) hides behind the next chunk's matmuls
        def g1(ct):
            # SP is idle in the shared phase: xt loads and zt stores go
            # there, leaving the scalar engine free for silu + PSUM drains
            if ct + 2 < NCT:
                xt_tiles[ct + 2] = xs.tile([128, KD, TCHUNK], f16, tag="x",
                                           name=f"xt{ct + 2}")
                nc.sync.dma_start(xt_tiles[ct + 2][:], xt[ct + 2])
            xt_c = xt_tiles.pop(ct)
            hst = hstp.tile([128, KS, TCHUNK], f16, tag="hst", name=f"hst{ct}")
            for i in range(KS):
                p1 = pg12.tile([128, TCHUNK], f32, tag="p1", name="p1")
                p3 = pg12.tile([128, TCHUNK], f32, tag="p3", name="p3")
                for ko in range(KD):
                    nc.tensor.matmul(
                        p1[:], ws1r[i][:, ko * 128:(ko + 1) * 128],
                        xt_c[:, ko, :],
                        start=(ko == 0), stop=(ko == KD - 1))
                for ko in range(KD):
                    nc.tensor.matmul(
                        p3[:], ws3r[i][:, ko * 128:(ko + 1) * 128],
                        xt_c[:, ko, :],
                        start=(ko == 0), stop=(ko == KD - 1))
                a1 = acts.tile([128, TCHUNK], f16, tag="acts", name="a1")
                nc.scalar.activation(a1[:], p1[:], AF.Silu)
                nc.vector.tensor_mul(hst[:, i], a1[:], p3[:])
            return hst

        def g2(ct, hst):
            for half in range(2):
                ztst = stgs.tile([128, KD // 2, TCHUNK], f16, tag="stgs",
                                 name=f"ztst{ct}_{half}")
                last = (ct == NCT - 1 and half == 1)
                for dd in range(KD // 2):
                    d = half * (KD // 2) + dd
                    py = pg3.tile([128, TCHUNK], f32, tag="py", name="py")
                    w2half = ws2r[half]
                    for io in range(KS):
                        nc.tensor.matmul(
                            py[:], w2half[:, dd, io * 128:(io + 1) * 128],
                            hst[:, io],
                            start=(io == 0), stop=(io == KS - 1))
                    # alternate the PSUM->SBUF drain across two engines so
                    # it keeps up with the 3-matmul fill
                    if d % 2 == 0:
                        nc.scalar.activation(ztst[:, dd], py[:], AF.Copy)
                    else:
                        nc.vector.tensor_copy(ztst[:, dd], py[:])
                    if last and dd == 3:
                        # the kernel's very last store splits in two so its
                        # wire time overlaps the remaining drains instead of
                        # trailing the final matmul
                        nc.sync.dma_start(zt[ct, half, :, :4], ztst[:, :4])
                if last:
                    nc.sync.dma_start(zt[ct, half, :, 4:], ztst[:, 4:])
                else:
                    nc.sync.dma_start(zt[ct, half], ztst[:])

        hst_prev = None
        for ct in range(NCT):
            hst = g1(ct)
            if hst_prev is not None:
                g2(ct - 1, hst_prev)
            hst_prev = hst
        g2(NCT - 1, hst_prev)

        stgs.release()
        acts.release()
        hstp.release()
        stg.release()
        actp.release()
        gwp.release()
        htp.release()
        wg3.release()
        shres.release()
        pg3.release()
        pg12.release()
        xs.release()
        wg12.release()

    nc.compile()
    return nc


def _get_nc(caps):
    key = ("nc", caps)
    if key not in _CACHE:
        _CACHE[key] = _build_nc(caps)
    return _CACHE[key]


# ------------------------------------------------------------------ kernel --
def kernel(x, Wg, W1, W3, W2, Ws1, Ws3, Ws2):
    from concourse.bass_utils import run_bass_kernel_spmd

    x = np.asarray(x, dtype=np.float32)
    x2d = np.ascontiguousarray(x.reshape(T, D))
    Wg = np.asarray(Wg, dtype=np.float32)
    W1 = np.asarray(W1, dtype=np.float32)
    W3 = np.asarray(W3, dtype=np.float32)
    W2 = np.asarray(W2, dtype=np.float32)
    Ws1 = np.asarray(Ws1, dtype=np.float32)
    Ws3 = np.asarray(Ws3, dtype=np.float32)
    Ws2 = np.asarray(Ws2, dtype=np.float32)

    # ---- host routing + dispatch ----
    topi, weights = _route(x2d, Wg)
    flat_e = topi.ravel()
    flat_t = np.repeat(np.arange(T), TOPK)
    flat_w = weights.ravel()
    order = np.argsort(flat_e, kind="stable")
    se, st_, sw = flat_e[order], flat_t[order], flat_w[order]
    bounds = np.searchsorted(se, np.arange(E + 1))
    tok_of = [st_[bounds[e]:bounds[e + 1]] for e in range(E)]
    wt_of = [sw[bounds[e]:bounds[e + 1]] for e in range(E)]
    counts = np.array([len(t) for t in tok_of])

    caps = _pick_caps(counts)
    cap0, cap1 = caps
    # expert -> (core, slot): slot 0 gets the 8 largest, slot 1 the 8
    # smallest (largest paired with smallest so per-core work is even)
    ranks = np.argsort(-counts, kind="stable")
    assign = [[int(ranks[c]), int(ranks[2 * NCORES - 1 - c])]
              for c in range(NCORES)]

    truncated = False
    for e in range(E):
        cap_e = cap0 if any(a[0] == e for a in assign) else cap1
        if len(tok_of[e]) > cap_e:  # only possible at the MAX_CAP clamp
            keep = np.sort(np.argsort(-wt_of[e], kind="stable")[:cap_e])
            tok_of[e] = tok_of[e][keep]
            wt_of[e] = wt_of[e][keep]
            truncated = True

    # ---- build per-core input maps ----
    xt_full = x2d.T.reshape(KD, 128, T).transpose(1, 0, 2)  # [128, KD, T]
    xt_tiles = np.ascontiguousarray(
        np.stack([xt_full[:, :, i * TCHUNK:(i + 1) * TCHUNK]
                  for i in range(T // TCHUNK)])).astype(F16)

    in_maps = []
    for c in range(NCORES):
        exps = assign[c]
        xg_c = np.stack([_tile_xT(x2d[tok_of[e]], cap0) for e in exps])
        # epilogue scale: py = (A2*W2)@(A3*h) => multiply by gw/(A2*A3)
        gw_c = np.zeros((ELOC, 128, cap0), dtype=np.float32)
        for s, e in enumerate(exps):
            gw_c[s, :, :len(wt_of[e])] = wt_of[e][None, :] / (A2 * A3)
        w1_c = np.stack([_tile_kxm4(W1[e], A1) for e in exps])
        w3_c = np.stack([_tile_kxm4(W3[e], A3) for e in exps])
        w2_c = np.stack([_tile_kxm4(W2[e], A2) for e in exps])

        lo = c * SI_SHARD
        ws1_s = np.zeros((SI_PAD, D), dtype=np.float32)
        ws1_s[:SI_SHARD] = Ws1[lo:lo + SI_SHARD]
        ws3_s = np.zeros((SI_PAD, D), dtype=np.float32)
        ws3_s[:SI_SHARD] = Ws3[lo:lo + SI_SHARD]
        ws2_s = np.zeros((D, SI_PAD), dtype=np.float32)
        ws2_s[:, :SI_SHARD] = Ws2[:, lo:lo + SI_SHARD]

        in_maps.append({
            "xg": xg_c, "gw": gw_c, "w1": w1_c, "w3": w3_c, "w2": w2_c,
            "xt": xt_tiles,
            "ws1": _tile_kxm(ws1_s), "ws3": _tile_kxm(ws3_s),
            "ws2": _tile_kxm(ws2_s),
        })

    # ---- run on 8 cores ----
    shapes = {
        "xg": ((ELOC, 128, KD, cap0), F8), "gw": ((ELOC, 128, cap0), np.float32),
        "w1": ((ELOC, KI, 128, KD, 128), F8), "w3": ((ELOC, KI, 128, KD, 128), F8),
        "w2": ((ELOC, KD, 128, KI, 128), F8),
        "xt": ((T // TCHUNK, 128, KD, TCHUNK), F16),
        "ws1": ((KS, 128, KD * 128), F16), "ws3": ((KS, 128, KD * 128), F16),
        "ws2": ((KD, 128, KS * 128), F16),
    }
    for m in in_maps:
        for k, v in m.items():
            assert v.shape == shapes[k][0], (k, v.shape, shapes[k][0])
            assert v.dtype == shapes[k][1], (k, v.dtype)

    nc = _get_nc(caps)
    res = run_bass_kernel_spmd(nc, in_maps, core_ids=list(range(NCORES)))
    _CACHE["last_results"] = res

    # ---- combine on host ----
    # routed: yt[s, d, p, c] = w * Y[c, d*128+p]
    cat_tok = []
    cat_rows = []
    for c in range(NCORES):
        ytc = res.results[c]["yt"]  # [ELOC, 128, KD, cap0] fp16
        for s in range(ELOC):
            e = assign[c][s]
            n = len(tok_of[e])
            rows = ytc[s].transpose(2, 1, 0).reshape(cap0, D)[:n].astype(
                np.float32)  # [n, D]
            cat_tok.append(tok_of[e])
            cat_rows.append(rows)
    cat_tok = np.concatenate(cat_tok)
    cat_rows = np.concatenate(cat_rows, axis=0)
    if truncated:
        y = np.zeros((T, D), dtype=np.float32)
        np.add.at(y, cat_tok, cat_rows)
    else:
        # every token has exactly TOPK contributions
        order = np.argsort(cat_tok, kind="stable")
        y = cat_rows[order].reshape(T, TOPK, D).sum(axis=1)

    # shared: sum partials; zt[ct, half, p, dd, t'] = Z[t, d*128+p] with
    # d = half*8+dd, t = ct*TCHUNK+t'
    z_acc = res.results[0]["zt"].astype(np.float32)
    for c in range(1, NCORES):
        z_acc += res.results[c]["zt"].astype(np.float32)
    z = z_acc.transpose(1, 3, 2, 0, 4).reshape(D, T).T  # [T, D]

    return (y + z).reshape(1, T, D).astype(np.float32)
